# revision 2
# baseline (speedup 1.0000x reference)
"""Trainium2 Bass kernel for the LogicLayer (difflogic) problem.

out[i, o] = c0[o] + ca[o]*a + cb[o]*b + cab[o]*a*b
  with a = x[i, idx_a[o]], b = x[i, idx_b[o]],
  [c0, ca, cb, cab] = softmax(weights[o]) @ GATE_COEFFS.

Strategy (8 cores, batch-sharded, 512 rows/core), batch-major, bf16:
  - x shard resident in SBUF as [128, 4, 8192] bf16 (row = bb*128 + p).
  - coeffs resident in SBUF as [128, 4, 8192] bf16 (pre-broadcast on host).
  - gathers produce [128, 4(bb), OCHUNK] tiles with the output dim packed
    in the last axis (eligible for DVE 2x/4x perf modes):
      * Pool chunks: gpsimd.indirect_copy per bb plane (base-ucode op,
        runs on the GpSimd engine itself)
      * DMA chunks: gpsimd.dma_gather(transpose=True) from an HBM copy of
        x^T (SWDGE desc-gen on Pool, transfer on the 16 DMA engines)
  - combine with 6 DVE tensor_tensor ops per chunk using stride-0
    broadcast APs over the bb axis; store bf16 y; host casts to f32.
"""

import numpy as np
from ml_dtypes import bfloat16

BATCH, IN_DIM, OUT_DIM = 4096, 8192, 8192
N_CORES = 8
ROWS = BATCH // N_CORES  # 512 rows per core
P = 128
N_BB = ROWS // P         # 4 batch blocks per core
OCHUNK = 1024            # output columns per chunk
N_CHUNK = OUT_DIM // OCHUNK
K_POOL = 3               # chunks gathered on Pool via indirect_copy

GATE_COEFFS = np.array([
    [0, 0, 0, 0], [0, 0, 0, 1], [0, 1, 0, -1], [0, 1, 0, 0],
    [0, 0, 1, -1], [0, 0, 1, 0], [0, 1, 1, -2], [0, 1, 1, -1],
    [1, -1, -1, 1], [1, -1, -1, 2], [1, 0, -1, 0], [1, 0, -1, 1],
    [1, -1, 0, 0], [1, -1, 0, 1], [1, 0, 0, -1], [1, 0, 0, 0],
], dtype=np.float32)  # [16, 4]

_CACHE = {}


def _build_nc(n_reps=1):
    import concourse.bacc as bacc
    import concourse.mybir as mybir
    from concourse.tile import TileContext

    bf16 = mybir.dt.bfloat16
    i16 = mybir.dt.int16
    u16 = mybir.dt.uint16

    nc = bacc.Bacc("TRN2", target_bir_lowering=False, debug=False,
                   num_devices=N_CORES)
    x = nc.dram_tensor("x", [P, N_BB, IN_DIM], bf16,
                       kind="ExternalInput").ap()
    xt = nc.dram_tensor("xt", [IN_DIM, ROWS], bf16,
                        kind="ExternalInput").ap()
    idxw = nc.dram_tensor("idxw", [P, OUT_DIM // 8], i16,
                          kind="ExternalInput").ap()
    cbt = nc.dram_tensor("cbt", [P, 4, OUT_DIM], bf16,
                         kind="ExternalInput").ap()
    y = nc.dram_tensor("y", [ROWS, OUT_DIM], bf16, kind="ExternalOutput").ap()

    y_t = y.rearrange("(bb p) m -> p bb m", p=P)      # [128, 4, 8192]
    icols = OCHUNK // 16  # idx columns per chunk
    half = OUT_DIM // 16  # start of idx_b block in idxw

    with TileContext(nc) as tc:
        with tc.tile_pool(name="xr", bufs=1) as xpool, \
             tc.tile_pool(name="const", bufs=1) as cpool:
            xa = xpool.tile([P, N_BB, IN_DIM], bf16, tag="xa")
            nc.sync.dma_start(out=xa[:], in_=x)
            co = cpool.tile([P, 4, OUT_DIM], bf16, tag="co")
            nc.sync.dma_start(out=co[:], in_=cbt)
            idx_sb = cpool.tile([P, OUT_DIM // 8], i16, tag="idx")
            nc.sync.dma_start(out=idx_sb[:], in_=idxw)

            for rep in range(n_reps):
                with tc.tile_pool(name=f"ab{rep}", bufs=2) as abpool, \
                     tc.tile_pool(name=f"qr{rep}", bufs=2) as qpool:
                    for c in range(N_CHUNK):
                        sl = slice(c * OCHUNK, (c + 1) * OCHUNK)
                        ia = idx_sb[:, c * icols:(c + 1) * icols]
                        ib = idx_sb[:, half + c * icols:half + (c + 1) * icols]
                        ga = abpool.tile([P, N_BB, OCHUNK], bf16, tag="a")
                        gb = abpool.tile([P, N_BB, OCHUNK], bf16, tag="b")
                        if c % N_CHUNK < K_POOL:
                            for bb in range(N_BB):
                                nc.gpsimd.indirect_copy(
                                    ga[:, bb, :], xa[:, bb, :],
                                    ia.bitcast(u16), True)
                                nc.gpsimd.indirect_copy(
                                    gb[:, bb, :], xa[:, bb, :],
                                    ib.bitcast(u16), True)
                        else:
                            nc.gpsimd.dma_gather(
                                out_ap=ga[:], in_ap=xt, idxs_ap=ia,
                                num_idxs=OCHUNK, num_idxs_reg=OCHUNK,
                                elem_size=ROWS, transpose=True)
                            nc.gpsimd.dma_gather(
                                out_ap=gb[:], in_ap=xt, idxs_ap=ib,
                                num_idxs=OCHUNK, num_idxs_reg=OCHUNK,
                                elem_size=ROWS, transpose=True)
                        shp = [P, N_BB, OCHUNK]
                        c0 = co[:, 0:1, sl].broadcast_to(shp)
                        ca = co[:, 1:2, sl].broadcast_to(shp)
                        cb = co[:, 2:3, sl].broadcast_to(shp)
                        cab = co[:, 3:4, sl].broadcast_to(shp)
                        q = qpool.tile(shp, bf16, tag="q")
                        # q = (a*cab + cb) * b
                        nc.vector.tensor_mul(q[:], ga[:], cab)
                        nc.vector.tensor_add(q[:], q[:], cb)
                        nc.vector.tensor_mul(q[:], q[:], gb[:])
                        # r = a*ca + c0 (in-place in ga)
                        nc.vector.tensor_mul(ga[:], ga[:], ca)
                        nc.vector.tensor_add(ga[:], ga[:], c0)
                        # out = q + r
                        nc.vector.tensor_add(q[:], q[:], ga[:])
                        nc.sync.dma_start(out=y_t[:, :, sl], in_=q[:])
    nc.compile()
    return nc


def _prep_host(x, weights, idx_a, idx_b):
    x = np.asarray(x, dtype=np.float32)
    w = np.asarray(weights, dtype=np.float32)
    e = np.exp(w - w.max(axis=1, keepdims=True))
    sm = e / e.sum(axis=1, keepdims=True)
    coeffs = (sm @ GATE_COEFFS).astype(np.float32)          # [8192, 4]
    cbt = np.ascontiguousarray(
        np.broadcast_to(coeffs.T[None, :, :], (P, 4, OUT_DIM))
    ).astype(bfloat16)                                       # [128, 4, 8192]
    ia = np.asarray(idx_a).astype(np.int16)
    ib = np.asarray(idx_b).astype(np.int16)

    def wrap(seq):  # j = s*16 + p16 -> [16, len/16] -> tile to 128 partitions
        m = seq.reshape(len(seq) // 16, 16).T
        return np.tile(m, (P // 16, 1))

    idxw = np.ascontiguousarray(
        np.concatenate([wrap(ia), wrap(ib)], axis=1))        # [128, 1024]
    xb = x.astype(bfloat16)
    xi = []
    for c in range(N_CORES):
        sh = xb[c * ROWS:(c + 1) * ROWS]         # [512, 8192] bf16
        xi.append({
            "x": np.ascontiguousarray(
                sh.reshape(N_BB, P, IN_DIM).transpose(1, 0, 2)),  # [128,4,8192]
            "xt": np.ascontiguousarray(sh.T),                     # [8192,512]
        })
    return xi, idxw, cbt


def _in_maps(x, weights, idx_a, idx_b):
    xi, idxw, cbt = _prep_host(x, weights, idx_a, idx_b)
    return [{"x": xi[c]["x"], "xt": xi[c]["xt"], "idxw": idxw, "cbt": cbt}
            for c in range(N_CORES)]


def kernel(x, weights, idx_a, idx_b):
    from concourse.bass_utils import run_bass_kernel_spmd

    in_maps = _in_maps(x, weights, idx_a, idx_b)
    if "nc" not in _CACHE:
        _CACHE["nc"] = _build_nc()
    nc = _CACHE["nc"]
    res = run_bass_kernel_spmd(nc, in_maps, list(range(N_CORES)))
    out = np.concatenate([res.results[c]["y"] for c in range(N_CORES)], axis=0)
    return out.astype(np.float32)
